# revision 35
# baseline (speedup 1.0000x reference)
"""Multi-head attention (B=8, C=512, L=2048, H=8, D=64) on 8 TRN2 NeuronCores.

Sharding: pure batch-parallel - core b computes batch b end-to-end (qkv proj,
8 heads of attention, out proj). No collectives.

Per-core layout strategy (v2 - dual-engine softmax):
  - qkv projection with lhsT = w_qkv.T (host-transposed), rhs = x.
  - S^T = K^T Q  (keys on partitions) so the exp output is already the
    transposed P^T needed by the PV matmul, and no max-subtraction is needed
    (scores are ~N(0,1) after the 1/sqrt(D) scale, folded into exp's scale).
  - Heads are processed in pairs (2t, 2t+1) that live in partition halves
    0-63 / 64-127 of one qkv row-tile. The two K=64 S^T matmuls of a pair
    run CONCURRENTLY in the PE array (row groups 0-1 vs 2-3) and write the
    two 512-column halves of one [128, 1024] PSUM tile.
  - exp runs on TWO engines: by default one ScalarE instruction covers both
    heads; on a tunable subset of j-tiles the h1 half goes to VectorE
    (2-op custom DVE: quartic poly then ^8 - the DVE pipeline is capped at
    8 ALU ops/instruction so poly+3 squarings cannot fuse) while ScalarE
    does only the h0 half. The corresponding PV(h1) is deferred one
    iteration so the in-order PE queue never waits on the slower DVE exp.
    exp scale is 8 (not 16): scores stay within +-6.8 so score/8 is in the
    quartic's fit range.
  - PV uses lhsT = [V^T | ones] (65 columns): row 64 of the accumulator is
    the softmax denominator, computed for free.
  - V^T is computed directly from X (lhsT = X tiles), V is never materialized.
  - softmax normalize: only the psum evacuations happen at the pair
    boundary; den/rec (VectorE) and the broadcast (Pool) + multiply
    (VectorE) are deferred into the NEXT pair's early iterations, so they
    never sit ahead of the next pair's exps in the in-order queues.
    GpSimd executes ONLY PartitionBroadcast - every Pool op-type switch
    costs a ~7us microcode LIBRARY_RELOAD that dead-stops the pipeline.
  - output projection: bias-add fused into the VectorE psum evacuation
    (a K=1 bias matmul was tried and costs ~376ns/instruction on the PE -
    more than the VectorE add it saves).
  - i is processed in 512-wide chunks (outer loop) so each chunk of the
    output projection overlaps the next chunk's attention pass.
"""

import os
import sys

sys.path.insert(0, "/opt/trn_rl_repo")

import numpy as np
import ml_dtypes

import concourse.bass as bass
import concourse.tile as tile
from concourse import bacc, mybir
from concourse import bass_utils

# ---- custom DVE exp: p = poly4(v), then p^8 (v = S/8) ----------------------
from concourse.dve_spec import Spec, Src0, C0, C1, C2, One, sq, lower, _has_src1
import concourse.dve_ops as dve_ops
from concourse.dve_ops import DveOp
from concourse.dve_uop import DveOpSpec

# minimax-ish fit of 1+v+v^2(c0+c1 v+c2 v^2) ~ e^v on |v| <= 0.85
# (max rel err 4.2e-4 -> 3.3e-3 after ^8; scores to +-6.8 sigma covered)
EXP_C = (0.50168003, 0.17185385, 0.03959494)


def _register_dve_op(name, spec):
    if name in dve_ops._SUB_OPCODE_FOR_NAME:
        return next(op for op in dve_ops.OPS if op.name == name)
    row = max(dve_ops._SUB_OPCODE_FOR_NAME.values()) + 1
    assert row < 0x20
    dve_ops._SUB_OPCODE_FOR_NAME[name] = row
    shas = {}
    for ver in ("v3", "v4"):
        s = DveOpSpec(
            name=name, opcode=row, uops=lower(spec, ver=ver), rd1_en=_has_src1(spec)
        )
        shas[ver] = s.sha(ver)
    op = DveOp(name, spec, subdim=False, uops_sha=shas)
    dve_ops.OPS.append(op)
    dve_ops.CUSTOM_DVE_SPECS[name] = spec
    return op


def _make_exp_ops():
    t = sq(Src0)
    spec1 = Spec(
        body=(One + Src0) + t * (C0 + C1 * Src0 + C2 * t),
        reference=lambda in0, in1, s0, s1, imm2: (
            1.0 + in0 + in0 * in0 * (s0 + s1 * in0 + imm2 * in0 * in0)
        ).astype(np.float32),
    )
    spec2 = Spec(
        body=sq(sq(sq(Src0))),
        reference=lambda in0, in1, s0, s1, imm2: (in0**8).astype(np.float32),
    )
    return (
        _register_dve_op("EXP8_POLY_ANT", spec1),
        _register_dve_op("POW8_ANT", spec2),
    )


EXP8_POLY, POW8 = _make_exp_ops()

B, C, L = 8, 512, 2048
H, D = 8, 64
HID = H * D  # 512
SCALE = float(D) ** -0.5
BF16 = mybir.dt.bfloat16
F32 = mybir.dt.float32
AF = mybir.ActivationFunctionType
NCORES = 8

NT = C // 128  # 4 channel tiles
NL = L // 512  # 4 l-chunks of 512
NJ = L // 128  # 16 key tiles

# j-tiles whose h1-half exp runs on VectorE instead of ScalarE.
# Strictly alternating so the Vector queue keeps up (a DVE exp costs ~2
# iterations of Vector time); jt 15 must stay on ScalarE (its PV cannot
# defer past the pair boundary).
# Default OFF: on full-speed silicon the kernel is TensorE-bound (PE ~297us
# busy vs ScalarE ~281us), so offloading exp to VectorE only adds coupling
# overhead (+25us measured). On parts/states where ScalarE is the slow
# engine the offload wins big (449us vs 516us with "1,3,5,7,9,11") - set
# KERNEL_DVE_JTS to enable.
_DVE_DEFAULT = ""
DVE_JTS = frozenset(
    int(x)
    for x in os.environ.get("KERNEL_DVE_JTS", _DVE_DEFAULT).split(",")
    if x != ""
) - {NJ - 1}
# split non-offload exps into h0/h1 halves (PV(h0) issues earlier)
SPLIT_EXP = bool(int(os.environ.get("KERNEL_SPLIT_EXP", "0")))


def build_kernel(tc, out_d, x_d, wqkvT_d, woutT_d, bias_d):
    nc = tc.nc
    from contextlib import ExitStack

    ctx = ExitStack()
    pers = ctx.enter_context(tc.tile_pool(name="pers", bufs=1))
    ptp = ctx.enter_context(tc.tile_pool(name="ptp", bufs=3))
    scrp = ctx.enter_context(tc.tile_pool(name="scrp", bufs=2))
    ytp = ctx.enter_context(tc.tile_pool(name="ytp", bufs=3))
    smp = ctx.enter_context(tc.tile_pool(name="smp", bufs=3))
    stp = ctx.enter_context(tc.tile_pool(name="stp", bufs=2, space="PSUM"))
    otp = ctx.enter_context(tc.tile_pool(name="otp", bufs=1, space="PSUM"))
    qkp = ctx.enter_context(tc.tile_pool(name="qkp", bufs=2, space="PSUM"))

    # ---- persistent SBUF tensors ----
    x_sb = [pers.tile([128, L], BF16, tag=f"x{c}", name=f"x{c}") for c in range(NT)]
    wq_sb = [
        pers.tile([128, 3 * HID], BF16, tag=f"wq{c}", name=f"wq{c}") for c in range(NT)
    ]
    wo_sb = [pers.tile([128, C], BF16, tag=f"wo{c}", name=f"wo{c}") for c in range(NT)]
    bias_sb = [
        pers.tile([128, 1], F32, tag=f"bias{c}", name=f"bias{c}") for c in range(NT)
    ]
    q_sb = [pers.tile([128, L], BF16, tag=f"q{t}", name=f"q{t}") for t in range(NT)]
    k_sb = [pers.tile([128, L], BF16, tag=f"k{t}", name=f"k{t}") for t in range(NT)]
    vt1 = [
        pers.tile([128, H * 65], BF16, tag=f"vt{j}", name=f"vt{j}") for j in range(NJ)
    ]
    o2 = [pers.tile([128, L], BF16, tag=f"o2_{c}", name=f"o2_{c}") for c in range(NT)]

    # ---- input DMAs, two waves on three trigger queues (Sync/Scalar/GpSimd).
    # Wave 1 is exactly what the first q/k projection groups and first V^T
    # tiles need (~0.8MB) so the first exp is not gated by the full 3.5MB
    # input load; wave 2 streams the rest behind it. ----
    for c in range(NT):
        r = slice(128 * c, 128 * (c + 1))
        nc.sync.dma_start(x_sb[c][:, 0:512], x_d[r, 0:512])
        nc.scalar.dma_start(wq_sb[c][:, 0:128], wqkvT_d[r, 0:128])
        nc.gpsimd.dma_start(wq_sb[c][:, 512:640], wqkvT_d[r, 512:640])
    for c in range(NT):
        r = slice(128 * c, 128 * (c + 1))
        nc.gpsimd.dma_start(wq_sb[c][:, 1024:1536], wqkvT_d[r, 1024:1536])
    for c in range(NT):
        r = slice(128 * c, 128 * (c + 1))
        nc.sync.dma_start(x_sb[c][:, 512:1024], x_d[r, 512:1024])
        nc.sync.dma_start(x_sb[c][:, 1024:1536], x_d[r, 1024:1536])
        nc.sync.dma_start(x_sb[c][:, 1536:2048], x_d[r, 1536:2048])
        nc.scalar.dma_start(wq_sb[c][:, 128:512], wqkvT_d[r, 128:512])
        nc.scalar.dma_start(wq_sb[c][:, 640:1024], wqkvT_d[r, 640:1024])

    # ---- PE warm-up: dummy matmuls during the input-DMA window so the HAM
    # clock gate opens (1.2 -> 2.4 GHz) before the real work arrives. The
    # chain ends in a DMA to an internal DRAM scratch so DCE keeps it. ----
    warm_scratch = nc.dram_tensor("warm_scratch", [128, 512], F32)
    warm_sb = pers.tile([128, 512], BF16, tag="warm", name="warm_sb")
    warm_out = pers.tile([128, 512], F32, tag="warmo", name="warm_out")
    nc.vector.memset(warm_sb[:, :], 0.001)
    wps = qkp.tile([128, 512], F32, tag="qkp", name="warm_ps")
    for w in range(18):
        nc.tensor.matmul(
            wps[:, :], lhsT=warm_sb[:, 0:128], rhs=warm_sb[:, :],
            start=True, stop=True,
        )
    nc.vector.tensor_copy(warm_out[:, :], wps[:, :])
    nc.sync.dma_start(warm_scratch.ap()[:, :], warm_out[:, :])

    def emit_qk_group(t, kind, n):
        """One projection psum group: q (kind=0) or k (kind=1) rows
        128t..128t+128 (heads 2t, 2t+1), l-chunk n. Lands directly in
        q_sb/k_sb (head 2t on partitions 0-63, head 2t+1 on 64-127)."""
        dst = (q_sb, k_sb)[kind][t]
        ocol = kind * HID + 128 * t
        ps = qkp.tile([128, 512], F32, tag="qkp", name=f"qk_ps_{kind}_{t}_{n}")
        for c in range(NT):
            nc.tensor.matmul(
                ps[:, :],
                lhsT=wq_sb[c][:, ocol : ocol + 128],
                rhs=x_sb[c][:, 512 * n : 512 * (n + 1)],
                start=(c == 0),
                stop=(c == NT - 1),
            )
        nc.vector.tensor_copy(dst[:, 512 * n : 512 * (n + 1)], ps[:, :])

    def emit_vt(jt):
        """V^T tile for key-block jt: [128 keys, 8 heads x (64 dims + ones)]."""
        ps = qkp.tile([128, 512], F32, tag="qkp", name=f"vt_ps_{jt}")
        for c in range(NT):
            nc.tensor.matmul(
                ps[:, :],
                lhsT=x_sb[c][:, 128 * jt : 128 * (jt + 1)],
                rhs=wq_sb[c][:, 2 * HID : 3 * HID],
                start=(c == 0),
                stop=(c == NT - 1),
            )
        vv = vt1[jt].rearrange("p (h e) -> p h e", e=65)
        nc.vector.tensor_copy(vv[:, :, 0:64], ps.rearrange("p (h d) -> p h d", d=64))
        nc.vector.memset(vv[:, :, 64:65], 1.0)

    def emit_st_for(t, ic, jt):
        islice = slice(512 * ic, 512 * ic + 512)
        jslice = slice(128 * jt, 128 * (jt + 1))
        st = stp.tile([128, 1024], F32, tag="st", name=f"st_{t}_{ic}_{jt}")
        # the two K=64 matmuls run concurrently (PE row groups 0-1 / 2-3)
        nc.tensor.matmul(
            st[:, 0:512], lhsT=k_sb[t][0:64, jslice], rhs=q_sb[t][0:64, islice],
            start=True, stop=True,
        )
        nc.tensor.matmul(
            st[:, 512:1024], lhsT=k_sb[t][64:128, jslice],
            rhs=q_sb[t][64:128, islice],
            start=True, stop=True,
        )
        return st

    def emit_pair(t, ic, interleave, vt_jit=False, first_st=None, next_ti=None,
                  dve_jts=frozenset(), post=None, inter_from=0, last=False):
        """Attention for head pair (2t, 2t+1), i-chunk ic (512 queries).
        `interleave` closures emit independent PE work into the loop; with
        vt_jit the V^T tiles are emitted just-in-time ahead of the PV that
        first needs them. `first_st` is this pair's S^T(0) if the previous
        pair already emitted it (cross-pair pipelining); if `next_ti` is
        given, the NEXT pair's S^T(0) is emitted BEFORE the last PVs.
        h1-half exps for jt in `dve_jts` run on VectorE (2-op poly^8) with
        their PV deferred one iteration. `post` holds the PREVIOUS pair's
        deferred normalize stages. Returns (next pair's S^T(0), this pair's
        deferred normalize stages)."""
        h0, h1 = 2 * t, 2 * t + 1
        ib = 512 * ic
        islice = slice(ib, ib + 512)
        ot0 = otp.tile([65, 512], F32, tag="ot0", name=f"ot0_{t}_{ic}")
        ot1 = otp.tile([65, 512], F32, tag="ot1", name=f"ot1_{t}_{ic}")

        pv_cnt = [0, 0]

        def emit_pv(hx, jt, pt):
            pv_cnt[hx] += 1
            ot = (ot0, ot1)[hx]
            h = (h0, h1)[hx]
            vt = vt1[jt]
            nc.tensor.matmul(
                ot[:, :], lhsT=vt[:, 65 * h : 65 * h + 65], rhs=pt[:, :],
                start=(jt == 0), stop=(pv_cnt[hx] == NJ),
            )

        slot = 0
        deferred = []
        next_first = None
        sts = {0: first_st if first_st is not None else emit_st_for(t, ic, 0)}
        for jt in range(NJ):
            st = sts.pop(jt)
            use_dve = jt in dve_jts
            if use_dve:
                # ScalarE covers h0 only (so PV(h0) is ready early);
                # VectorE computes h1 via (poly4)^8 on score/8.
                pt0 = ptp.tile([128, 512], BF16, tag="pt0",
                               name=f"pt0_{t}_{ic}_{jt}")
                nc.scalar.activation(pt0[:, :], st[:, 0:512], AF.Exp, scale=8.0)
                p1 = scrp.tile([128, 512], F32, tag="p1", name=f"p1_{t}_{ic}_{jt}")
                pt1 = ptp.tile([128, 512], BF16, tag="pt1",
                               name=f"pt1_{t}_{ic}_{jt}")
                nc.vector._custom_dve(
                    EXP8_POLY, out=p1[:, :], in0=st[:, 512:1024],
                    s0=EXP_C[0], s1=EXP_C[1], imm2=EXP_C[2],
                )
                nc.vector._custom_dve(POW8, out=pt1[:, :], in0=p1[:, :])
            elif SPLIT_EXP:
                # two ScalarE instructions, h0 first: PV(h0) can issue ~550ns
                # earlier than with one full-tile exp (it waits only on the
                # h0 half). Costs ~90ns/iter of ScalarE instruction overhead.
                ptf = ptp.tile([128, 1024], BF16, tag="ptf",
                               name=f"ptf_{t}_{ic}_{jt}")
                nc.scalar.activation(ptf[:, 0:512], st[:, 0:512], AF.Exp, scale=8.0)
                nc.scalar.activation(ptf[:, 512:1024], st[:, 512:1024], AF.Exp,
                                     scale=8.0)
                pt0, pt1 = ptf[:, 0:512], ptf[:, 512:1024]
            else:
                ptf = ptp.tile([128, 1024], BF16, tag="ptf",
                               name=f"ptf_{t}_{ic}_{jt}")
                nc.scalar.activation(ptf[:, :], st[:, :], AF.Exp, scale=8.0)
                pt0, pt1 = ptf[:, 0:512], ptf[:, 512:1024]
            if jt + 1 < NJ:
                sts[jt + 1] = emit_st_for(t, ic, jt + 1)
            elif next_ti is not None:
                # cross-pair: next pair's S^T(0) goes ahead of this pair's
                # last PVs in the PE stream
                next_first = emit_st_for(next_ti[0], next_ti[1], 0)
            # V^T tiles emitted in-loop so they never gate the first exp;
            # >=2-iteration lead keeps their DVE copies off PV's critical path
            if vt_jit:
                if jt == 0:
                    emit_vt(0)
                    emit_vt(1)
                    emit_vt(2)
                elif jt + 2 < NJ:
                    emit_vt(jt + 2)
            # previous pair's deferred normalize stages, one per iteration
            if post is not None and 1 <= jt <= len(post):
                post[jt - 1]()
            # fill PE slack with independent work, spread across the loop,
            # and emitted BEFORE this iteration's PVs: PV(h0) waits on the
            # exp (~190ns on ~40% of iterations), and a proj/qk matmul
            # placed ahead of it in the in-order PE stream absorbs that
            # wait with useful work.
            # Proj groups read o2 written by the previous pair's deferred
            # muls (injected at jt 3-4), so for ic>=1 the slots start at
            # jt=5 - an interleave group emitted before its o2 writer would
            # read stale data (the framework can't wait on a future writer).
            if inter_from == 0:
                target = ((jt + 1) * len(interleave) + 11) // 12
            elif jt < inter_from:
                target = 0
            else:
                target = ((jt - inter_from + 1) * len(interleave) + 10) // 11
            while slot < min(target, len(interleave)):
                interleave[slot]()
                slot += 1
            # a DVE-produced pt1 arrives ~1us later than a ScalarE one; its
            # PV would head-of-line-block the in-order PE queue, so defer it
            # one iteration (accumulation order within ot1 is preserved).
            while deferred and deferred[0][0] <= jt - 1:
                emit_pv(1, *deferred.pop(0))
            emit_pv(0, jt, pt0)
            if use_dve or SPLIT_EXP:
                # with SPLIT_EXP every PV(h1) defers one iteration: pt1 is
                # produced second on ScalarE, so its PV would otherwise wait
                # ~600ns at the head of the in-order PE queue
                deferred.append((jt, pt1))
            else:
                emit_pv(1, jt, pt1)
        for djt, dpt in deferred:
            emit_pv(1, djt, dpt)
        # softmax normalization: divide rows 0-63 by the ones-row (64).
        # Only the psum evacuations happen here (they gate the next pair's
        # first PVs via the ot pool); den/rec/broadcast/mul are deferred
        # into the next pair via `post` - emitted here they would block the
        # next pair's DVE exps in the in-order Vector queue for ~5us.
        # reciprocal_approx_fast mis-reads non-zero partition offsets on
        # silicon, so the denominator row is staged to partition 0 first.
        if last:
            # no next pair contends for the ot psum banks: skip the o2u
            # staging copies and normalize straight out of PSUM (saves
            # ~1.3us off the end-of-kernel critical chain)
            den0 = smp.tile([1, 512], F32, tag="den0", name=f"den_{h0}_{ic}")
            nc.vector.tensor_copy(den0[:, :], ot0[64:65, :])
            rec0 = smp.tile([1, 512], F32, tag="rec0", name=f"rec_{h0}_{ic}")
            nc.vector.reciprocal_approx_fast(rec0[:, :], den0[:, :])
            den1 = smp.tile([1, 512], F32, tag="den1", name=f"den_{h1}_{ic}")
            nc.vector.tensor_copy(den1[:, :], ot1[64:65, :])
            rec1 = smp.tile([1, 512], F32, tag="rec1", name=f"rec_{h1}_{ic}")
            nc.vector.reciprocal_approx_fast(rec1[:, :], den1[:, :])
            rb0 = smp.tile([64, 512], F32, tag="rb0", name=f"rb_{h0}_{ic}")
            nc.gpsimd.partition_broadcast(rb0[:, :], rec0[:, :])
            nc.vector.tensor_mul(o2[t][0:64, islice], ot0[0:64, :], rb0[:, :])
            rb1 = smp.tile([64, 512], F32, tag="rb1", name=f"rb_{h1}_{ic}")
            nc.gpsimd.partition_broadcast(rb1[:, :], rec1[:, :])
            nc.vector.tensor_mul(o2[t][64:128, islice], ot1[0:64, :], rb1[:, :])
            return next_first, []
        o2u0 = smp.tile([65, 512], F32, tag="o2u0", name=f"o2u_{h0}_{ic}")
        nc.vector.tensor_copy(o2u0[:, :], ot0[:, :])
        o2u1 = smp.tile([65, 512], F32, tag="o2u1", name=f"o2u_{h1}_{ic}")
        nc.vector.tensor_copy(o2u1[:, :], ot1[:, :])
        den0 = smp.tile([1, 512], F32, tag="den0", name=f"den_{h0}_{ic}")
        rec0 = smp.tile([1, 512], F32, tag="rec0", name=f"rec_{h0}_{ic}")
        den1 = smp.tile([1, 512], F32, tag="den1", name=f"den_{h1}_{ic}")
        rec1 = smp.tile([1, 512], F32, tag="rec1", name=f"rec_{h1}_{ic}")
        rb0 = smp.tile([64, 512], F32, tag="rb0", name=f"rb_{h0}_{ic}")
        rb1 = smp.tile([64, 512], F32, tag="rb1", name=f"rb_{h1}_{ic}")

        def post_den0():
            nc.vector.tensor_copy(den0[:, :], o2u0[64:65, :])
            nc.vector.reciprocal_approx_fast(rec0[:, :], den0[:, :])

        def post_den1():
            nc.vector.tensor_copy(den1[:, :], o2u1[64:65, :])
            nc.vector.reciprocal_approx_fast(rec1[:, :], den1[:, :])

        def post_rb():
            # Pool runs ONLY PartitionBroadcast (op-type switches cost a
            # ~7us microcode reload on the Q7)
            nc.gpsimd.partition_broadcast(rb0[:, :], rec0[:, :])
            nc.gpsimd.partition_broadcast(rb1[:, :], rec1[:, :])

        def post_mul():
            nc.vector.tensor_mul(o2[t][0:64, islice], o2u0[0:64, :], rb0[:, :])
            nc.vector.tensor_mul(o2[t][64:128, islice], o2u1[0:64, :], rb1[:, :])

        return next_first, [post_den0, post_den1, post_rb, post_mul]

    held_proj = {}

    def emit_proj_group(o, n, c_lo=0):
        if c_lo == 0:
            ps = qkp.tile([128, 512], F32, tag="qkp", name=f"y_ps_{o}_{n}")
        else:
            ps = held_proj.pop((o, n))
        for c in range(c_lo, NT):
            nc.tensor.matmul(
                ps[:, :],
                lhsT=wo_sb[c][:, 128 * o : 128 * (o + 1)],
                rhs=o2[c][:, 512 * n : 512 * (n + 1)],
                start=(c == 0),
                stop=(c == NT - 1),
            )
        yt = ytp.tile([128, 512], F32, tag="yt", name=f"yt_{o}_{n}")
        nc.vector.tensor_scalar_add(yt[:, :], ps[:, :], bias_sb[o][:, 0:1])
        nc.sync.dma_start(
            out_d[128 * o : 128 * (o + 1), 512 * n : 512 * (n + 1)], yt[:, :]
        )

    def emit_proj_partial(o, n):
        """First 3 channel-tiles of proj group (o, n); the psum tile is held
        and finished by emit_proj_group(o, n, c_lo=3) once the last pair's
        output is ready."""
        ps = qkp.tile([128, 512], F32, tag="qkp", name=f"y_ps_{o}_{n}")
        for c in range(3):
            nc.tensor.matmul(
                ps[:, :],
                lhsT=wo_sb[c][:, 128 * o : 128 * (o + 1)],
                rhs=o2[c][:, 512 * n : 512 * (n + 1)],
                start=(c == 0),
                stop=False,
            )
        held_proj[(o, n)] = ps

    # ---- emission schedule ----
    # pair 0's q (chunk 0) + full k projected up front; everything else is
    # interleaved just-in-time into earlier attention loops.
    emit_qk_group(0, 0, 0)
    emit_qk_group(0, 1, 0)

    # wo/bias loads off the critical startup path
    for c in range(NT):
        r = slice(128 * c, 128 * (c + 1))
        nc.sync.dma_start(wo_sb[c][:, :], woutT_d[r, :])
        nc.sync.dma_start(bias_sb[c][:, :], bias_d[r, :])

    def kg(t, n):
        return lambda: emit_qk_group(t, 1, n)

    def qg(t, n):
        return lambda: emit_qk_group(t, 0, n)

    def pj(o, n):
        return lambda: emit_proj_group(o, n)

    # pair t's q chunk for pass ic must be emitted BEFORE its (ic, t) loop
    # (the PE executes in order - a dependency later in its own stream would
    # deadlock). q chunks for pass ic+1 therefore fire during pass ic, and
    # proj chunk n fires during pass n+1.
    inter = {
        (0, 0): [kg(0, 1), kg(0, 2), kg(0, 3), qg(1, 0), kg(1, 0)],
        (0, 1): [kg(1, 1), kg(1, 2), kg(1, 3), qg(2, 0), kg(2, 0)],
        (0, 2): [kg(2, 1), kg(2, 2), kg(2, 3), qg(3, 0), kg(3, 0), qg(0, 1)],
        (0, 3): [kg(3, 1), kg(3, 2), kg(3, 3), qg(1, 1), qg(2, 1), qg(3, 1)],
        (1, 0): [qg(0, 2), pj(0, 0)],
        (1, 1): [qg(1, 2), pj(1, 0)],
        (1, 2): [qg(2, 2), pj(2, 0)],
        (1, 3): [qg(3, 2), pj(3, 0)],
        (2, 0): [qg(0, 3), pj(0, 1)],
        (2, 1): [qg(1, 3), pj(1, 1)],
        (2, 2): [qg(2, 3), pj(2, 1)],
        (2, 3): [qg(3, 3), pj(3, 1)],
        (3, 0): [pj(0, 2)],
        (3, 1): [pj(1, 2)],
        (3, 2): [pj(2, 2), pj(3, 2)],
        # only 2 partials fit: qkp has 2 psum banks and each held partial
        # pins one until its c=3 finisher pops it
        (3, 3): [
            lambda: emit_proj_partial(0, 3),
            lambda: emit_proj_partial(1, 3),
        ],
    }
    seq = [(ic, t) for ic in range(4) for t in range(NT)]
    pending_st = None
    pending_post = None
    for i, (ic, t) in enumerate(seq):
        nxt = seq[i + 1] if i + 1 < len(seq) else None
        # no DVE offload in the very first pair (its DVE queue is busy with
        # JIT V^T evacuations) and none on jt 15 (enforced in DVE_JTS)
        dj = frozenset() if (ic == 0 and t == 0) else DVE_JTS
        pending_st, pending_post = emit_pair(
            t, ic, inter.get((ic, t), []),
            vt_jit=(ic == 0 and t == 0),
            first_st=pending_st,
            next_ti=(nxt[1], nxt[0]) if nxt else None,
            dve_jts=dj,
            post=pending_post,
            inter_from=0 if ic == 0 else 5,
            last=(i + 1 == len(seq)),
        )
    for p in pending_post:
        p()
    emit_proj_group(0, 3, c_lo=3)
    emit_proj_group(1, 3, c_lo=3)
    emit_proj_group(2, 3)
    emit_proj_group(3, 3)
    ctx.close()


_COMPILED = None


def _get_compiled():
    global _COMPILED
    if _COMPILED is None:
        nc = bacc.Bacc(
            "TRN2", target_bir_lowering=False, debug=False, num_devices=NCORES
        )
        x_d = nc.dram_tensor("x", [C, L], BF16, kind="ExternalInput").ap()
        wqkvT_d = nc.dram_tensor("wqkvT", [C, 3 * HID], BF16, kind="ExternalInput").ap()
        woutT_d = nc.dram_tensor("woutT", [HID, C], BF16, kind="ExternalInput").ap()
        bias_d = nc.dram_tensor("bias", [C, 1], F32, kind="ExternalInput").ap()
        out_d = nc.dram_tensor("out", [C, L], F32, kind="ExternalOutput").ap()
        with tile.TileContext(nc) as tc:
            build_kernel(tc, out_d, x_d, wqkvT_d, woutT_d, bias_d)
        nc.compile()
        _COMPILED = nc
    return _COMPILED


def make_in_maps(x, w_qkv, w_out, b_out):
    xb = np.asarray(x, dtype=np.float32).astype(ml_dtypes.bfloat16)
    wq_f = np.asarray(w_qkv, dtype=np.float32).T.copy()
    wq_f[:, 0:HID] *= SCALE / 8.0  # exp scale folded into the q projection
    wqkvT = np.ascontiguousarray(wq_f.astype(ml_dtypes.bfloat16))
    woutT = np.ascontiguousarray(
        np.asarray(w_out, dtype=np.float32).T.astype(ml_dtypes.bfloat16)
    )
    bias = np.ascontiguousarray(np.asarray(b_out, dtype=np.float32).reshape(C, 1))
    return [
        {
            "x": np.ascontiguousarray(xb[b]),
            "wqkvT": wqkvT,
            "woutT": woutT,
            "bias": bias,
        }
        for b in range(B)
    ]


LAST_RESULTS = None


def _install_ntff_hook():
    """Provide antenv.axon_hooks (absent from this image) so trace=True works."""
    import types

    try:
        from antenv.axon_hooks import get_axon_ntff_profile_hook  # noqa: F401

        return
    except ImportError:
        pass
    sys.path.insert(0, "/root/.axon_site")
    from trn_agent_boot.trn_boot import _ntff_profile_via_ctypes

    hook = _ntff_profile_via_ctypes("/opt/axon/libaxon_pjrt.so")
    import antenv

    mod = types.ModuleType("antenv.axon_hooks")
    mod._hook = hook
    mod.get_axon_ntff_profile_hook = lambda: mod._hook
    mod.set_axon_ntff_profile_hook = lambda h: setattr(mod, "_hook", h)
    sys.modules["antenv.axon_hooks"] = mod
    antenv.axon_hooks = mod
    # artifact upload has no egress in this container - make it a no-op
    bass_utils.upload_artifacts = lambda tmpdir: tmpdir


def kernel(x, w_qkv, w_out, b_out):
    global LAST_RESULTS
    nc = _get_compiled()
    in_maps = make_in_maps(x, w_qkv, w_out, b_out)
    trace = bool(int(os.environ.get("KERNEL_TRACE", "0")))
    if trace:
        _install_ntff_hook()
    res = bass_utils.run_bass_kernel_spmd(
        nc, in_maps, core_ids=list(range(NCORES)), trace=trace
    )
    LAST_RESULTS = res
    out = np.stack([np.asarray(res.results[b]["out"]) for b in range(B)])
    return out.astype(np.float32)


# revision 42
# speedup vs baseline: 1.0009x; 1.0009x over previous
"""Multi-head attention (B=8, C=512, L=2048, H=8, D=64) on 8 TRN2 NeuronCores.

Sharding: pure batch-parallel - core b computes batch b end-to-end (qkv proj,
8 heads of attention, out proj). No collectives.

Per-core layout strategy (v2 - dual-engine softmax):
  - qkv projection with lhsT = w_qkv.T (host-transposed), rhs = x.
  - S^T = K^T Q  (keys on partitions) so the exp output is already the
    transposed P^T needed by the PV matmul, and no max-subtraction is needed
    (scores are ~N(0,1) after the 1/sqrt(D) scale, folded into exp's scale).
  - Heads are processed in pairs (2t, 2t+1) that live in partition halves
    0-63 / 64-127 of one qkv row-tile. The two K=64 S^T matmuls of a pair
    run CONCURRENTLY in the PE array (row groups 0-1 vs 2-3) and write the
    two 512-column halves of one [128, 1024] PSUM tile.
  - exp runs on TWO engines: by default one ScalarE instruction covers both
    heads; on a tunable subset of j-tiles the h1 half goes to VectorE
    (2-op custom DVE: quartic poly then ^8 - the DVE pipeline is capped at
    8 ALU ops/instruction so poly+3 squarings cannot fuse) while ScalarE
    does only the h0 half. The corresponding PV(h1) is deferred one
    iteration so the in-order PE queue never waits on the slower DVE exp.
    exp scale is 8 (not 16): scores stay within +-6.8 so score/8 is in the
    quartic's fit range.
  - PV uses lhsT = [V^T | ones] (65 columns): row 64 of the accumulator is
    the softmax denominator, computed for free.
  - V^T is computed directly from X (lhsT = X tiles), V is never materialized.
  - softmax normalize: only the psum evacuations happen at the pair
    boundary; den/rec (VectorE) and the broadcast (Pool) + multiply
    (VectorE) are deferred into the NEXT pair's early iterations, so they
    never sit ahead of the next pair's exps in the in-order queues.
    GpSimd executes ONLY PartitionBroadcast - every Pool op-type switch
    costs a ~7us microcode LIBRARY_RELOAD that dead-stops the pipeline.
  - output projection: bias-add fused into the VectorE psum evacuation
    (a K=1 bias matmul was tried and costs ~376ns/instruction on the PE -
    more than the VectorE add it saves).
  - i is processed in 512-wide chunks (outer loop) so each chunk of the
    output projection overlaps the next chunk's attention pass.
"""

import os
import sys

sys.path.insert(0, "/opt/trn_rl_repo")

import numpy as np
import ml_dtypes

import concourse.bass as bass
import concourse.tile as tile
from concourse import bacc, mybir
from concourse import bass_utils

# ---- custom DVE exp: p = poly4(v), then p^8 (v = S/8) ----------------------
from concourse.dve_spec import Spec, Src0, C0, C1, C2, One, sq, lower, _has_src1
import concourse.dve_ops as dve_ops
from concourse.dve_ops import DveOp
from concourse.dve_uop import DveOpSpec

# minimax-ish fit of 1+v+v^2(c0+c1 v+c2 v^2) ~ e^v on |v| <= 0.85
# (max rel err 4.2e-4 -> 3.3e-3 after ^8; scores to +-6.8 sigma covered)
EXP_C = (0.50168003, 0.17185385, 0.03959494)


def _register_dve_op(name, spec):
    if name in dve_ops._SUB_OPCODE_FOR_NAME:
        return next(op for op in dve_ops.OPS if op.name == name)
    row = max(dve_ops._SUB_OPCODE_FOR_NAME.values()) + 1
    assert row < 0x20
    dve_ops._SUB_OPCODE_FOR_NAME[name] = row
    shas = {}
    for ver in ("v3", "v4"):
        s = DveOpSpec(
            name=name, opcode=row, uops=lower(spec, ver=ver), rd1_en=_has_src1(spec)
        )
        shas[ver] = s.sha(ver)
    op = DveOp(name, spec, subdim=False, uops_sha=shas)
    dve_ops.OPS.append(op)
    dve_ops.CUSTOM_DVE_SPECS[name] = spec
    return op


def _make_exp_ops():
    t = sq(Src0)
    spec1 = Spec(
        body=(One + Src0) + t * (C0 + C1 * Src0 + C2 * t),
        reference=lambda in0, in1, s0, s1, imm2: (
            1.0 + in0 + in0 * in0 * (s0 + s1 * in0 + imm2 * in0 * in0)
        ).astype(np.float32),
    )
    spec2 = Spec(
        body=sq(sq(sq(Src0))),
        reference=lambda in0, in1, s0, s1, imm2: (in0**8).astype(np.float32),
    )
    return (
        _register_dve_op("EXP8_POLY_ANT", spec1),
        _register_dve_op("POW8_ANT", spec2),
    )


EXP8_POLY, POW8 = _make_exp_ops()

B, C, L = 8, 512, 2048
H, D = 8, 64
HID = H * D  # 512
SCALE = float(D) ** -0.5
BF16 = mybir.dt.bfloat16
F32 = mybir.dt.float32
AF = mybir.ActivationFunctionType
NCORES = 8

NT = C // 128  # 4 channel tiles
NL = L // 512  # 4 l-chunks of 512
NJ = L // 128  # 16 key tiles

# j-tiles whose h1-half exp runs on VectorE instead of ScalarE.
# Strictly alternating so the Vector queue keeps up (a DVE exp costs ~2
# iterations of Vector time); jt 15 must stay on ScalarE (its PV cannot
# defer past the pair boundary).
# Default OFF: on full-speed silicon the kernel is TensorE-bound (PE ~297us
# busy vs ScalarE ~281us), so offloading exp to VectorE only adds coupling
# overhead (+25us measured). On parts/states where ScalarE is the slow
# engine the offload wins big (449us vs 516us with "1,3,5,7,9,11") - set
# KERNEL_DVE_JTS to enable.
_DVE_DEFAULT = ""
DVE_JTS = frozenset(
    int(x)
    for x in os.environ.get("KERNEL_DVE_JTS", _DVE_DEFAULT).split(",")
    if x != ""
) - {NJ - 1}
# split non-offload exps into h0/h1 halves (PV(h0) issues earlier)
SPLIT_EXP = bool(int(os.environ.get("KERNEL_SPLIT_EXP", "0")))
# proj-psum evacuation engine: "v" = VectorE tensor_scalar_add, "s" =
# ScalarE Identity+bias (Identity shares the exp_and_others ACT table, so
# no table reload; needs ScalarE slack -> pair with a small DVE offload)
YT_ENGINE = os.environ.get("KERNEL_YT", "v")


def build_kernel(tc, out_d, x_d, wqkvT_d, woutT_d, bias_d):
    nc = tc.nc
    from contextlib import ExitStack

    ctx = ExitStack()
    pers = ctx.enter_context(tc.tile_pool(name="pers", bufs=1))
    ptp = ctx.enter_context(tc.tile_pool(name="ptp", bufs=3))
    scrp = ctx.enter_context(tc.tile_pool(name="scrp", bufs=2))
    ytp = ctx.enter_context(tc.tile_pool(name="ytp", bufs=3))
    smp = ctx.enter_context(tc.tile_pool(name="smp", bufs=3))
    stp = ctx.enter_context(tc.tile_pool(name="stp", bufs=2, space="PSUM"))
    otp = ctx.enter_context(tc.tile_pool(name="otp", bufs=1, space="PSUM"))
    qkp = ctx.enter_context(tc.tile_pool(name="qkp", bufs=2, space="PSUM"))

    # ---- persistent SBUF tensors ----
    x_sb = [pers.tile([128, L], BF16, tag=f"x{c}", name=f"x{c}") for c in range(NT)]
    wq_sb = [
        pers.tile([128, 3 * HID], BF16, tag=f"wq{c}", name=f"wq{c}") for c in range(NT)
    ]
    wo_sb = [pers.tile([128, C], BF16, tag=f"wo{c}", name=f"wo{c}") for c in range(NT)]
    bias_sb = [
        pers.tile([128, 1], F32, tag=f"bias{c}", name=f"bias{c}") for c in range(NT)
    ]
    q_sb = [pers.tile([128, L], BF16, tag=f"q{t}", name=f"q{t}") for t in range(NT)]
    k_sb = [pers.tile([128, L], BF16, tag=f"k{t}", name=f"k{t}") for t in range(NT)]
    vt1 = [
        pers.tile([128, H * 65], BF16, tag=f"vt{j}", name=f"vt{j}") for j in range(NJ)
    ]
    o2 = [pers.tile([128, L], BF16, tag=f"o2_{c}", name=f"o2_{c}") for c in range(NT)]

    # ---- input DMAs, two waves on three trigger queues (Sync/Scalar/GpSimd).
    # Wave 1 is exactly what the first q/k projection groups and first V^T
    # tiles need (~0.8MB) so the first exp is not gated by the full 3.5MB
    # input load; wave 2 streams the rest behind it. ----
    for c in range(NT):
        r = slice(128 * c, 128 * (c + 1))
        nc.sync.dma_start(x_sb[c][:, 0:512], x_d[r, 0:512])
        nc.scalar.dma_start(wq_sb[c][:, 0:128], wqkvT_d[r, 0:128])
        nc.gpsimd.dma_start(wq_sb[c][:, 512:640], wqkvT_d[r, 512:640])
    for c in range(NT):
        r = slice(128 * c, 128 * (c + 1))
        nc.gpsimd.dma_start(wq_sb[c][:, 1024:1536], wqkvT_d[r, 1024:1536])
    for c in range(NT):
        r = slice(128 * c, 128 * (c + 1))
        nc.sync.dma_start(x_sb[c][:, 512:1024], x_d[r, 512:1024])
        nc.sync.dma_start(x_sb[c][:, 1024:1536], x_d[r, 1024:1536])
        nc.sync.dma_start(x_sb[c][:, 1536:2048], x_d[r, 1536:2048])
        nc.scalar.dma_start(wq_sb[c][:, 128:512], wqkvT_d[r, 128:512])
        nc.scalar.dma_start(wq_sb[c][:, 640:1024], wqkvT_d[r, 640:1024])

    # ---- PE warm-up: dummy matmuls during the input-DMA window so the HAM
    # clock gate opens (1.2 -> 2.4 GHz) before the real work arrives. The
    # chain ends in a DMA to an internal DRAM scratch so DCE keeps it. ----
    warm_scratch = nc.dram_tensor("warm_scratch", [128, 512], F32)
    warm_sb = pers.tile([128, 512], BF16, tag="warm", name="warm_sb")
    warm_out = pers.tile([128, 512], F32, tag="warmo", name="warm_out")
    nc.vector.memset(warm_sb[:, :], 0.001)
    wps = qkp.tile([128, 512], F32, tag="qkp", name="warm_ps")
    for w in range(18):
        nc.tensor.matmul(
            wps[:, :], lhsT=warm_sb[:, 0:128], rhs=warm_sb[:, :],
            start=True, stop=True,
        )
    nc.vector.tensor_copy(warm_out[:, :], wps[:, :])
    nc.sync.dma_start(warm_scratch.ap()[:, :], warm_out[:, :])

    held_qk = {}

    def emit_qk_group(t, kind, n, c_lo=0, c_hi=NT):
        """One projection psum group: q (kind=0) or k (kind=1) rows
        128t..128t+128 (heads 2t, 2t+1), l-chunk n. Lands directly in
        q_sb/k_sb (head 2t on partitions 0-63, head 2t+1 on 64-127).
        May be emitted in two phases (c_lo/c_hi) so interleave filler
        spreads across twice as many attention iterations."""
        dst = (q_sb, k_sb)[kind][t]
        ocol = kind * HID + 128 * t
        if c_lo == 0:
            ps = qkp.tile([128, 512], F32, tag="qkp", name=f"qk_ps_{kind}_{t}_{n}")
        else:
            ps = held_qk.pop((kind, t, n))
        for c in range(c_lo, c_hi):
            nc.tensor.matmul(
                ps[:, :],
                lhsT=wq_sb[c][:, ocol : ocol + 128],
                rhs=x_sb[c][:, 512 * n : 512 * (n + 1)],
                start=(c == 0),
                stop=(c == NT - 1),
            )
        if c_hi < NT:
            held_qk[(kind, t, n)] = ps
            return
        nc.vector.tensor_copy(dst[:, 512 * n : 512 * (n + 1)], ps[:, :])

    def emit_vt(jt):
        """V^T tile for key-block jt: [128 keys, 8 heads x (64 dims + ones)]."""
        ps = qkp.tile([128, 512], F32, tag="qkp", name=f"vt_ps_{jt}")
        for c in range(NT):
            nc.tensor.matmul(
                ps[:, :],
                lhsT=x_sb[c][:, 128 * jt : 128 * (jt + 1)],
                rhs=wq_sb[c][:, 2 * HID : 3 * HID],
                start=(c == 0),
                stop=(c == NT - 1),
            )
        vv = vt1[jt].rearrange("p (h e) -> p h e", e=65)
        nc.vector.tensor_copy(vv[:, :, 0:64], ps.rearrange("p (h d) -> p h d", d=64))
        nc.vector.memset(vv[:, :, 64:65], 1.0)

    def emit_st_for(t, ic, jt):
        islice = slice(512 * ic, 512 * ic + 512)
        jslice = slice(128 * jt, 128 * (jt + 1))
        st = stp.tile([128, 1024], F32, tag="st", name=f"st_{t}_{ic}_{jt}")
        # the two K=64 matmuls run concurrently (PE row groups 0-1 / 2-3)
        nc.tensor.matmul(
            st[:, 0:512], lhsT=k_sb[t][0:64, jslice], rhs=q_sb[t][0:64, islice],
            start=True, stop=True,
        )
        nc.tensor.matmul(
            st[:, 512:1024], lhsT=k_sb[t][64:128, jslice],
            rhs=q_sb[t][64:128, islice],
            start=True, stop=True,
        )
        return st

    def emit_pair(t, ic, interleave, vt_jit=False, first_st=None, next_ti=None,
                  dve_jts=frozenset(), post=None, inter_from=0, last=False):
        """Attention for head pair (2t, 2t+1), i-chunk ic (512 queries).
        `interleave` closures emit independent PE work into the loop; with
        vt_jit the V^T tiles are emitted just-in-time ahead of the PV that
        first needs them. `first_st` is this pair's S^T(0) if the previous
        pair already emitted it (cross-pair pipelining); if `next_ti` is
        given, the NEXT pair's S^T(0) is emitted BEFORE the last PVs.
        h1-half exps for jt in `dve_jts` run on VectorE (2-op poly^8) with
        their PV deferred one iteration. `post` holds the PREVIOUS pair's
        deferred normalize stages. Returns (next pair's S^T(0), this pair's
        deferred normalize stages)."""
        h0, h1 = 2 * t, 2 * t + 1
        ib = 512 * ic
        islice = slice(ib, ib + 512)
        ot0 = otp.tile([65, 512], F32, tag="ot0", name=f"ot0_{t}_{ic}")
        ot1 = otp.tile([65, 512], F32, tag="ot1", name=f"ot1_{t}_{ic}")

        pv_cnt = [0, 0]

        def emit_pv(hx, jt, pt):
            pv_cnt[hx] += 1
            ot = (ot0, ot1)[hx]
            h = (h0, h1)[hx]
            vt = vt1[jt]
            nc.tensor.matmul(
                ot[:, :], lhsT=vt[:, 65 * h : 65 * h + 65], rhs=pt[:, :],
                start=(jt == 0), stop=(pv_cnt[hx] == NJ),
            )

        slot = 0
        deferred = []
        next_first = None
        sts = {0: first_st if first_st is not None else emit_st_for(t, ic, 0)}
        for jt in range(NJ):
            st = sts.pop(jt)
            use_dve = jt in dve_jts
            if use_dve:
                # ScalarE covers h0 only (so PV(h0) is ready early);
                # VectorE computes h1 via (poly4)^8 on score/8.
                pt0 = ptp.tile([128, 512], BF16, tag="pt0",
                               name=f"pt0_{t}_{ic}_{jt}")
                nc.scalar.activation(pt0[:, :], st[:, 0:512], AF.Exp, scale=8.0)
                p1 = scrp.tile([128, 512], F32, tag="p1", name=f"p1_{t}_{ic}_{jt}")
                pt1 = ptp.tile([128, 512], BF16, tag="pt1",
                               name=f"pt1_{t}_{ic}_{jt}")
                nc.vector._custom_dve(
                    EXP8_POLY, out=p1[:, :], in0=st[:, 512:1024],
                    s0=EXP_C[0], s1=EXP_C[1], imm2=EXP_C[2],
                )
                nc.vector._custom_dve(POW8, out=pt1[:, :], in0=p1[:, :])
            elif SPLIT_EXP:
                # two ScalarE instructions, h0 first: PV(h0) can issue ~550ns
                # earlier than with one full-tile exp (it waits only on the
                # h0 half). Costs ~90ns/iter of ScalarE instruction overhead.
                ptf = ptp.tile([128, 1024], BF16, tag="ptf",
                               name=f"ptf_{t}_{ic}_{jt}")
                nc.scalar.activation(ptf[:, 0:512], st[:, 0:512], AF.Exp, scale=8.0)
                nc.scalar.activation(ptf[:, 512:1024], st[:, 512:1024], AF.Exp,
                                     scale=8.0)
                pt0, pt1 = ptf[:, 0:512], ptf[:, 512:1024]
            else:
                ptf = ptp.tile([128, 1024], BF16, tag="ptf",
                               name=f"ptf_{t}_{ic}_{jt}")
                nc.scalar.activation(ptf[:, :], st[:, :], AF.Exp, scale=8.0)
                pt0, pt1 = ptf[:, 0:512], ptf[:, 512:1024]
            if jt + 1 < NJ:
                sts[jt + 1] = emit_st_for(t, ic, jt + 1)
            elif next_ti is not None:
                # cross-pair: next pair's S^T(0) goes ahead of this pair's
                # last PVs in the PE stream
                next_first = emit_st_for(next_ti[0], next_ti[1], 0)
            # V^T tiles emitted in-loop so they never gate the first exp;
            # >=2-iteration lead keeps their DVE copies off PV's critical path
            if vt_jit:
                if jt == 0:
                    emit_vt(0)
                    emit_vt(1)
                    emit_vt(2)
                elif jt + 2 < NJ:
                    emit_vt(jt + 2)
            # previous pair's deferred normalize stages, one per iteration
            if post is not None and 1 <= jt <= len(post):
                post[jt - 1]()
            # fill PE slack with independent work, spread across the loop,
            # and emitted BEFORE this iteration's PVs: PV(h0) waits on the
            # exp (~190ns on ~40% of iterations), and a proj/qk matmul
            # placed ahead of it in the in-order PE stream absorbs that
            # wait with useful work.
            # Proj groups read o2 written by the previous pair's deferred
            # muls (injected at jt 3-4), so for ic>=1 the slots start at
            # jt=5 - an interleave group emitted before its o2 writer would
            # read stale data (the framework can't wait on a future writer).
            if inter_from == 0:
                target = ((jt + 1) * len(interleave) + 11) // 12
            elif jt < inter_from:
                target = 0
            else:
                target = ((jt - inter_from + 1) * len(interleave) + 10) // 11
            while slot < min(target, len(interleave)):
                interleave[slot]()
                slot += 1
            # a DVE-produced pt1 arrives ~1us later than a ScalarE one; its
            # PV would head-of-line-block the in-order PE queue, so defer it
            # one iteration (accumulation order within ot1 is preserved).
            while deferred and deferred[0][0] <= jt - 1:
                emit_pv(1, *deferred.pop(0))
            emit_pv(0, jt, pt0)
            if use_dve or SPLIT_EXP:
                # with SPLIT_EXP every PV(h1) defers one iteration: pt1 is
                # produced second on ScalarE, so its PV would otherwise wait
                # ~600ns at the head of the in-order PE queue
                deferred.append((jt, pt1))
            else:
                emit_pv(1, jt, pt1)
        for djt, dpt in deferred:
            emit_pv(1, djt, dpt)
        # softmax normalization: divide rows 0-63 by the ones-row (64).
        # Only the psum evacuations happen here (they gate the next pair's
        # first PVs via the ot pool); den/rec/broadcast/mul are deferred
        # into the next pair via `post` - emitted here they would block the
        # next pair's DVE exps in the in-order Vector queue for ~5us.
        # reciprocal_approx_fast mis-reads non-zero partition offsets on
        # silicon, so the denominator row is staged to partition 0 first.
        if last:
            # no next pair contends for the ot psum banks: skip the o2u
            # staging copies and normalize straight out of PSUM (saves
            # ~1.3us off the end-of-kernel critical chain)
            den0 = smp.tile([1, 512], F32, tag="den0", name=f"den_{h0}_{ic}")
            nc.vector.tensor_copy(den0[:, :], ot0[64:65, :])
            rec0 = smp.tile([1, 512], F32, tag="rec0", name=f"rec_{h0}_{ic}")
            nc.vector.reciprocal_approx_fast(rec0[:, :], den0[:, :])
            den1 = smp.tile([1, 512], F32, tag="den1", name=f"den_{h1}_{ic}")
            nc.vector.tensor_copy(den1[:, :], ot1[64:65, :])
            rec1 = smp.tile([1, 512], F32, tag="rec1", name=f"rec_{h1}_{ic}")
            nc.vector.reciprocal_approx_fast(rec1[:, :], den1[:, :])
            rb0 = smp.tile([64, 512], F32, tag="rb0", name=f"rb_{h0}_{ic}")
            nc.gpsimd.partition_broadcast(rb0[:, :], rec0[:, :])
            nc.vector.tensor_mul(o2[t][0:64, islice], ot0[0:64, :], rb0[:, :])
            rb1 = smp.tile([64, 512], F32, tag="rb1", name=f"rb_{h1}_{ic}")
            nc.gpsimd.partition_broadcast(rb1[:, :], rec1[:, :])
            nc.vector.tensor_mul(o2[t][64:128, islice], ot1[0:64, :], rb1[:, :])
            return next_first, []
        o2u0 = smp.tile([65, 512], F32, tag="o2u0", name=f"o2u_{h0}_{ic}")
        nc.vector.tensor_copy(o2u0[:, :], ot0[:, :])
        o2u1 = smp.tile([65, 512], F32, tag="o2u1", name=f"o2u_{h1}_{ic}")
        nc.vector.tensor_copy(o2u1[:, :], ot1[:, :])
        den0 = smp.tile([1, 512], F32, tag="den0", name=f"den_{h0}_{ic}")
        rec0 = smp.tile([1, 512], F32, tag="rec0", name=f"rec_{h0}_{ic}")
        den1 = smp.tile([1, 512], F32, tag="den1", name=f"den_{h1}_{ic}")
        rec1 = smp.tile([1, 512], F32, tag="rec1", name=f"rec_{h1}_{ic}")
        rb0 = smp.tile([64, 512], F32, tag="rb0", name=f"rb_{h0}_{ic}")
        rb1 = smp.tile([64, 512], F32, tag="rb1", name=f"rb_{h1}_{ic}")

        def post_den0():
            nc.vector.tensor_copy(den0[:, :], o2u0[64:65, :])
            nc.vector.reciprocal_approx_fast(rec0[:, :], den0[:, :])

        def post_den1():
            nc.vector.tensor_copy(den1[:, :], o2u1[64:65, :])
            nc.vector.reciprocal_approx_fast(rec1[:, :], den1[:, :])

        def post_rb():
            # Pool runs ONLY PartitionBroadcast (op-type switches cost a
            # ~7us microcode reload on the Q7)
            nc.gpsimd.partition_broadcast(rb0[:, :], rec0[:, :])
            nc.gpsimd.partition_broadcast(rb1[:, :], rec1[:, :])

        def post_mul():
            nc.vector.tensor_mul(o2[t][0:64, islice], o2u0[0:64, :], rb0[:, :])
            nc.vector.tensor_mul(o2[t][64:128, islice], o2u1[0:64, :], rb1[:, :])

        return next_first, [post_den0, post_den1, post_rb, post_mul]

    held_proj = {}

    def emit_proj_group(o, n, c_lo=0, c_hi=NT):
        if c_lo == 0:
            ps = qkp.tile([128, 512], F32, tag="qkp", name=f"y_ps_{o}_{n}")
        else:
            ps = held_proj.pop((o, n))
        for c in range(c_lo, c_hi):
            nc.tensor.matmul(
                ps[:, :],
                lhsT=wo_sb[c][:, 128 * o : 128 * (o + 1)],
                rhs=o2[c][:, 512 * n : 512 * (n + 1)],
                start=(c == 0),
                stop=(c == NT - 1),
            )
        if c_hi < NT:
            held_proj[(o, n)] = ps
            return
        yt = ytp.tile([128, 512], F32, tag="yt", name=f"yt_{o}_{n}")
        if YT_ENGINE == "s":
            nc.scalar.activation(
                yt[:, :], ps[:, :], AF.Identity, bias=bias_sb[o][:, 0:1]
            )
        else:
            nc.vector.tensor_scalar_add(yt[:, :], ps[:, :], bias_sb[o][:, 0:1])
        nc.sync.dma_start(
            out_d[128 * o : 128 * (o + 1), 512 * n : 512 * (n + 1)], yt[:, :]
        )

    def emit_proj_partial(o, n):
        """First 3 channel-tiles of proj group (o, n); the psum tile is held
        and finished by emit_proj_group(o, n, c_lo=3) once the last pair's
        output is ready."""
        ps = qkp.tile([128, 512], F32, tag="qkp", name=f"y_ps_{o}_{n}")
        for c in range(3):
            nc.tensor.matmul(
                ps[:, :],
                lhsT=wo_sb[c][:, 128 * o : 128 * (o + 1)],
                rhs=o2[c][:, 512 * n : 512 * (n + 1)],
                start=(c == 0),
                stop=False,
            )
        held_proj[(o, n)] = ps

    # ---- emission schedule ----
    # pair 0's q (chunk 0) + full k projected up front; everything else is
    # interleaved just-in-time into earlier attention loops.
    emit_qk_group(0, 0, 0)
    emit_qk_group(0, 1, 0)

    # wo/bias loads off the critical startup path
    for c in range(NT):
        r = slice(128 * c, 128 * (c + 1))
        nc.sync.dma_start(wo_sb[c][:, :], woutT_d[r, :])
        nc.sync.dma_start(bias_sb[c][:, :], bias_d[r, :])

    # each builder returns a 1-element phase list; splitting groups into
    # two 2-matmul phases was tried (doubles the slots with PE filler) but
    # measured within noise of single-phase, which has more test mileage
    def kg(t, n):
        return [lambda: emit_qk_group(t, 1, n)]

    def qg(t, n):
        return [lambda: emit_qk_group(t, 0, n)]

    def pj(o, n):
        return [lambda: emit_proj_group(o, n)]

    # pair t's q chunk for pass ic must be emitted BEFORE its (ic, t) loop
    # (the PE executes in order - a dependency later in its own stream would
    # deadlock). q chunks for pass ic+1 therefore fire during pass ic, and
    # proj chunk n fires during pass n+1.
    inter = {
        (0, 0): kg(0, 1) + kg(0, 2) + kg(0, 3) + qg(1, 0) + kg(1, 0),
        (0, 1): kg(1, 1) + kg(1, 2) + kg(1, 3) + qg(2, 0) + kg(2, 0),
        (0, 2): kg(2, 1) + kg(2, 2) + kg(2, 3) + qg(3, 0) + kg(3, 0) + qg(0, 1),
        (0, 3): kg(3, 1) + kg(3, 2) + kg(3, 3) + qg(1, 1) + qg(2, 1) + qg(3, 1),
        (1, 0): qg(0, 2) + pj(0, 0),
        (1, 1): qg(1, 2) + pj(1, 0),
        (1, 2): qg(2, 2) + pj(2, 0),
        (1, 3): qg(3, 2) + pj(3, 0),
        (2, 0): qg(0, 3) + pj(0, 1),
        (2, 1): qg(1, 3) + pj(1, 1),
        (2, 2): qg(2, 3) + pj(2, 1),
        (2, 3): qg(3, 3) + pj(3, 1),
        (3, 0): pj(0, 2),
        (3, 1): pj(1, 2),
        (3, 2): pj(2, 2) + pj(3, 2),
        # only 2 partials fit: qkp has 2 psum banks and each held partial
        # pins one until its c=3 finisher pops it
        (3, 3): [
            lambda: emit_proj_partial(0, 3),
            lambda: emit_proj_partial(1, 3),
        ],
    }
    seq = [(ic, t) for ic in range(4) for t in range(NT)]
    pending_st = None
    pending_post = None
    for i, (ic, t) in enumerate(seq):
        nxt = seq[i + 1] if i + 1 < len(seq) else None
        # no DVE offload in the very first pair (its DVE queue is busy with
        # JIT V^T evacuations) and none on jt 15 (enforced in DVE_JTS)
        dj = frozenset() if (ic == 0 and t == 0) else DVE_JTS
        pending_st, pending_post = emit_pair(
            t, ic, inter.get((ic, t), []),
            vt_jit=(ic == 0 and t == 0),
            first_st=pending_st,
            next_ti=(nxt[1], nxt[0]) if nxt else None,
            dve_jts=dj,
            post=pending_post,
            inter_from=0 if ic == 0 else 5,
            last=(i + 1 == len(seq)),
        )
    for p in pending_post:
        p()
    emit_proj_group(0, 3, c_lo=3)
    emit_proj_group(1, 3, c_lo=3)
    emit_proj_group(2, 3)
    emit_proj_group(3, 3)
    ctx.close()


_COMPILED = None


def _get_compiled():
    global _COMPILED
    if _COMPILED is None:
        nc = bacc.Bacc(
            "TRN2", target_bir_lowering=False, debug=False, num_devices=NCORES
        )
        x_d = nc.dram_tensor("x", [C, L], BF16, kind="ExternalInput").ap()
        wqkvT_d = nc.dram_tensor("wqkvT", [C, 3 * HID], BF16, kind="ExternalInput").ap()
        woutT_d = nc.dram_tensor("woutT", [HID, C], BF16, kind="ExternalInput").ap()
        bias_d = nc.dram_tensor("bias", [C, 1], F32, kind="ExternalInput").ap()
        out_d = nc.dram_tensor("out", [C, L], F32, kind="ExternalOutput").ap()
        with tile.TileContext(nc) as tc:
            build_kernel(tc, out_d, x_d, wqkvT_d, woutT_d, bias_d)
        nc.compile()
        _COMPILED = nc
    return _COMPILED


def make_in_maps(x, w_qkv, w_out, b_out):
    xb = np.asarray(x, dtype=np.float32).astype(ml_dtypes.bfloat16)
    wq_f = np.asarray(w_qkv, dtype=np.float32).T.copy()
    wq_f[:, 0:HID] *= SCALE / 8.0  # exp scale folded into the q projection
    wqkvT = np.ascontiguousarray(wq_f.astype(ml_dtypes.bfloat16))
    woutT = np.ascontiguousarray(
        np.asarray(w_out, dtype=np.float32).T.astype(ml_dtypes.bfloat16)
    )
    bias = np.ascontiguousarray(np.asarray(b_out, dtype=np.float32).reshape(C, 1))
    return [
        {
            "x": np.ascontiguousarray(xb[b]),
            "wqkvT": wqkvT,
            "woutT": woutT,
            "bias": bias,
        }
        for b in range(B)
    ]


LAST_RESULTS = None


def _install_ntff_hook():
    """Provide antenv.axon_hooks (absent from this image) so trace=True works."""
    import types

    try:
        from antenv.axon_hooks import get_axon_ntff_profile_hook  # noqa: F401

        return
    except ImportError:
        pass
    sys.path.insert(0, "/root/.axon_site")
    from trn_agent_boot.trn_boot import _ntff_profile_via_ctypes

    hook = _ntff_profile_via_ctypes("/opt/axon/libaxon_pjrt.so")
    import antenv

    mod = types.ModuleType("antenv.axon_hooks")
    mod._hook = hook
    mod.get_axon_ntff_profile_hook = lambda: mod._hook
    mod.set_axon_ntff_profile_hook = lambda h: setattr(mod, "_hook", h)
    sys.modules["antenv.axon_hooks"] = mod
    antenv.axon_hooks = mod
    # artifact upload has no egress in this container - make it a no-op
    bass_utils.upload_artifacts = lambda tmpdir: tmpdir


def kernel(x, w_qkv, w_out, b_out):
    global LAST_RESULTS
    nc = _get_compiled()
    in_maps = make_in_maps(x, w_qkv, w_out, b_out)
    trace = bool(int(os.environ.get("KERNEL_TRACE", "0")))
    if trace:
        _install_ntff_hook()
    res = bass_utils.run_bass_kernel_spmd(
        nc, in_maps, core_ids=list(range(NCORES)), trace=trace
    )
    LAST_RESULTS = res
    out = np.stack([np.asarray(res.results[b]["out"]) for b in range(B)])
    return out.astype(np.float32)


# revision 47
# speedup vs baseline: 1.0055x; 1.0046x over previous
"""Multi-head attention (B=8, C=512, L=2048, H=8, D=64) on 8 TRN2 NeuronCores.

Sharding: pure batch-parallel - core b computes batch b end-to-end (qkv proj,
8 heads of attention, out proj). No collectives.

Per-core layout strategy (v2 - dual-engine softmax):
  - qkv projection with lhsT = w_qkv.T (host-transposed), rhs = x.
  - S^T = K^T Q  (keys on partitions) so the exp output is already the
    transposed P^T needed by the PV matmul, and no max-subtraction is needed
    (scores are ~N(0,1) after the 1/sqrt(D) scale, folded into exp's scale).
  - Heads are processed in pairs (2t, 2t+1) that live in partition halves
    0-63 / 64-127 of one qkv row-tile. The two K=64 S^T matmuls of a pair
    run CONCURRENTLY in the PE array (row groups 0-1 vs 2-3) and write the
    two 512-column halves of one [128, 1024] PSUM tile.
  - exp runs on TWO engines: by default one ScalarE instruction covers both
    heads; on a tunable subset of j-tiles the h1 half goes to VectorE
    (2-op custom DVE: quartic poly then ^8 - the DVE pipeline is capped at
    8 ALU ops/instruction so poly+3 squarings cannot fuse) while ScalarE
    does only the h0 half. The corresponding PV(h1) is deferred one
    iteration so the in-order PE queue never waits on the slower DVE exp.
    exp scale is 8 (not 16): scores stay within +-6.8 so score/8 is in the
    quartic's fit range.
  - PV uses lhsT = [V^T | ones] (65 columns): row 64 of the accumulator is
    the softmax denominator, computed for free.
  - V^T is computed directly from X (lhsT = X tiles), V is never materialized.
  - softmax normalize: only the psum evacuations happen at the pair
    boundary; den/rec (VectorE) and the broadcast (Pool) + multiply
    (VectorE) are deferred into the NEXT pair's early iterations, so they
    never sit ahead of the next pair's exps in the in-order queues.
    GpSimd executes ONLY PartitionBroadcast - every Pool op-type switch
    costs a ~7us microcode LIBRARY_RELOAD that dead-stops the pipeline.
  - output projection: bias-add fused into the VectorE psum evacuation
    (a K=1 bias matmul was tried and costs ~376ns/instruction on the PE -
    more than the VectorE add it saves).
  - i is processed in 512-wide chunks (outer loop) so each chunk of the
    output projection overlaps the next chunk's attention pass.
"""

import os
import sys

sys.path.insert(0, "/opt/trn_rl_repo")

import numpy as np
import ml_dtypes

import concourse.bass as bass
import concourse.tile as tile
from concourse import bacc, mybir
from concourse import bass_utils

# ---- custom DVE exp: p = poly4(v), then p^8 (v = S/8) ----------------------
from concourse.dve_spec import Spec, Src0, C0, C1, C2, One, sq, lower, _has_src1
import concourse.dve_ops as dve_ops
from concourse.dve_ops import DveOp
from concourse.dve_uop import DveOpSpec

# minimax-ish fit of 1+v+v^2(c0+c1 v+c2 v^2) ~ e^v on |v| <= 0.85
# (max rel err 4.2e-4 -> 3.3e-3 after ^8; scores to +-6.8 sigma covered)
EXP_C = (0.50168003, 0.17185385, 0.03959494)


def _register_dve_op(name, spec):
    if name in dve_ops._SUB_OPCODE_FOR_NAME:
        return next(op for op in dve_ops.OPS if op.name == name)
    row = max(dve_ops._SUB_OPCODE_FOR_NAME.values()) + 1
    assert row < 0x20
    dve_ops._SUB_OPCODE_FOR_NAME[name] = row
    shas = {}
    for ver in ("v3", "v4"):
        s = DveOpSpec(
            name=name, opcode=row, uops=lower(spec, ver=ver), rd1_en=_has_src1(spec)
        )
        shas[ver] = s.sha(ver)
    op = DveOp(name, spec, subdim=False, uops_sha=shas)
    dve_ops.OPS.append(op)
    dve_ops.CUSTOM_DVE_SPECS[name] = spec
    return op


def _make_exp_ops():
    t = sq(Src0)
    spec1 = Spec(
        body=(One + Src0) + t * (C0 + C1 * Src0 + C2 * t),
        reference=lambda in0, in1, s0, s1, imm2: (
            1.0 + in0 + in0 * in0 * (s0 + s1 * in0 + imm2 * in0 * in0)
        ).astype(np.float32),
    )
    spec2 = Spec(
        body=sq(sq(sq(Src0))),
        reference=lambda in0, in1, s0, s1, imm2: (in0**8).astype(np.float32),
    )
    return (
        _register_dve_op("EXP8_POLY_ANT", spec1),
        _register_dve_op("POW8_ANT", spec2),
    )


EXP8_POLY, POW8 = _make_exp_ops()

B, C, L = 8, 512, 2048
H, D = 8, 64
HID = H * D  # 512
SCALE = float(D) ** -0.5
BF16 = mybir.dt.bfloat16
F32 = mybir.dt.float32
AF = mybir.ActivationFunctionType
NCORES = 8

NT = C // 128  # 4 channel tiles
NL = L // 512  # 4 l-chunks of 512
NJ = L // 128  # 16 key tiles

# j-tiles whose h1-half exp runs on VectorE instead of ScalarE.
# Strictly alternating so the Vector queue keeps up (a DVE exp costs ~2
# iterations of Vector time); jt 15 must stay on ScalarE (its PV cannot
# defer past the pair boundary).
# Default OFF: on full-speed silicon the kernel is TensorE-bound (PE ~297us
# busy vs ScalarE ~281us), so offloading exp to VectorE only adds coupling
# overhead (+25us measured). On parts/states where ScalarE is the slow
# engine the offload wins big (449us vs 516us with "1,3,5,7,9,11") - set
# KERNEL_DVE_JTS to enable.
_DVE_DEFAULT = ""
DVE_JTS = frozenset(
    int(x)
    for x in os.environ.get("KERNEL_DVE_JTS", _DVE_DEFAULT).split(",")
    if x != ""
) - {NJ - 1}
# split non-offload exps into h0/h1 halves (PV(h0) issues earlier)
SPLIT_EXP = bool(int(os.environ.get("KERNEL_SPLIT_EXP", "0")))
# proj-psum evacuation engine: "v" = VectorE tensor_scalar_add, "s" =
# ScalarE Identity+bias (Identity shares the exp_and_others ACT table, so
# no table reload; needs ScalarE slack -> pair with a small DVE offload)
YT_ENGINE = os.environ.get("KERNEL_YT", "v")


def build_kernel(tc, out_d, x_d, wqkvT_d, woutT_d, bias_d):
    nc = tc.nc
    from contextlib import ExitStack

    ctx = ExitStack()
    pers = ctx.enter_context(tc.tile_pool(name="pers", bufs=1))
    ptp = ctx.enter_context(tc.tile_pool(name="ptp", bufs=3))
    scrp = ctx.enter_context(tc.tile_pool(name="scrp", bufs=2))
    ytp = ctx.enter_context(tc.tile_pool(name="ytp", bufs=3))
    smp = ctx.enter_context(tc.tile_pool(name="smp", bufs=3))
    stp = ctx.enter_context(tc.tile_pool(name="stp", bufs=2, space="PSUM"))
    otp = ctx.enter_context(tc.tile_pool(name="otp", bufs=1, space="PSUM"))
    qkp = ctx.enter_context(tc.tile_pool(name="qkp", bufs=2, space="PSUM"))

    # ---- persistent SBUF tensors ----
    x_sb = [pers.tile([128, L], BF16, tag=f"x{c}", name=f"x{c}") for c in range(NT)]
    wq_sb = [
        pers.tile([128, 3 * HID], BF16, tag=f"wq{c}", name=f"wq{c}") for c in range(NT)
    ]
    wo_sb = [pers.tile([128, C], BF16, tag=f"wo{c}", name=f"wo{c}") for c in range(NT)]
    bias_sb = [
        pers.tile([128, 1], F32, tag=f"bias{c}", name=f"bias{c}") for c in range(NT)
    ]
    q_sb = [pers.tile([128, L], BF16, tag=f"q{t}", name=f"q{t}") for t in range(NT)]
    k_sb = [pers.tile([128, L], BF16, tag=f"k{t}", name=f"k{t}") for t in range(NT)]
    vt1 = [
        pers.tile([128, H * 65], BF16, tag=f"vt{j}", name=f"vt{j}") for j in range(NJ)
    ]
    o2 = [pers.tile([128, L], BF16, tag=f"o2_{c}", name=f"o2_{c}") for c in range(NT)]

    # ---- input DMAs, two waves on three trigger queues (Sync/Scalar/GpSimd).
    # Wave 1 is exactly what the first q/k projection groups and first V^T
    # tiles need (~0.8MB) so the first exp is not gated by the full 3.5MB
    # input load; wave 2 streams the rest behind it. ----
    for c in range(NT):
        r = slice(128 * c, 128 * (c + 1))
        nc.sync.dma_start(x_sb[c][:, 0:512], x_d[r, 0:512])
        nc.scalar.dma_start(wq_sb[c][:, 0:128], wqkvT_d[r, 0:128])
        nc.gpsimd.dma_start(wq_sb[c][:, 512:640], wqkvT_d[r, 512:640])
    for c in range(NT):
        r = slice(128 * c, 128 * (c + 1))
        nc.gpsimd.dma_start(wq_sb[c][:, 1024:1536], wqkvT_d[r, 1024:1536])
    # (need-first fine-grained wq ordering and a second trigger queue were
    # tried for the ~7us of pass-0 DMA-wait stalls: both measured worse -
    # smaller transfers lose more to per-DMA overhead than the reorder buys)
    for c in range(NT):
        r = slice(128 * c, 128 * (c + 1))
        nc.sync.dma_start(x_sb[c][:, 512:1024], x_d[r, 512:1024])
        nc.sync.dma_start(x_sb[c][:, 1024:1536], x_d[r, 1024:1536])
        nc.sync.dma_start(x_sb[c][:, 1536:2048], x_d[r, 1536:2048])
        nc.scalar.dma_start(wq_sb[c][:, 128:512], wqkvT_d[r, 128:512])
        nc.scalar.dma_start(wq_sb[c][:, 640:1024], wqkvT_d[r, 640:1024])

    # ---- PE warm-up: dummy matmuls during the input-DMA window so the HAM
    # clock gate opens (1.2 -> 2.4 GHz) before the real work arrives. The
    # chain ends in a DMA to an internal DRAM scratch so DCE keeps it. ----
    warm_scratch = nc.dram_tensor("warm_scratch", [128, 512], F32)
    warm_sb = pers.tile([128, 512], BF16, tag="warm", name="warm_sb")
    warm_out = pers.tile([128, 512], F32, tag="warmo", name="warm_out")
    nc.vector.memset(warm_sb[:, :], 0.001)
    wps = qkp.tile([128, 512], F32, tag="qkp", name="warm_ps")
    for w in range(18):
        nc.tensor.matmul(
            wps[:, :], lhsT=warm_sb[:, 0:128], rhs=warm_sb[:, :],
            start=True, stop=True,
        )
    nc.vector.tensor_copy(warm_out[:, :], wps[:, :])
    nc.sync.dma_start(warm_scratch.ap()[:, :], warm_out[:, :])

    held_qk = {}

    def emit_qk_group(t, kind, n, c_lo=0, c_hi=NT):
        """One projection psum group: q (kind=0) or k (kind=1) rows
        128t..128t+128 (heads 2t, 2t+1), l-chunk n. Lands directly in
        q_sb/k_sb (head 2t on partitions 0-63, head 2t+1 on 64-127).
        May be emitted in two phases (c_lo/c_hi) so interleave filler
        spreads across twice as many attention iterations."""
        dst = (q_sb, k_sb)[kind][t]
        ocol = kind * HID + 128 * t
        if c_lo == 0:
            ps = qkp.tile([128, 512], F32, tag="qkp", name=f"qk_ps_{kind}_{t}_{n}")
        else:
            ps = held_qk.pop((kind, t, n))
        for c in range(c_lo, c_hi):
            nc.tensor.matmul(
                ps[:, :],
                lhsT=wq_sb[c][:, ocol : ocol + 128],
                rhs=x_sb[c][:, 512 * n : 512 * (n + 1)],
                start=(c == 0),
                stop=(c == NT - 1),
            )
        if c_hi < NT:
            held_qk[(kind, t, n)] = ps
            return
        nc.vector.tensor_copy(dst[:, 512 * n : 512 * (n + 1)], ps[:, :])

    def emit_vt(jt):
        """V^T tile for key-block jt: [128 keys, 8 heads x (64 dims + ones)]."""
        ps = qkp.tile([128, 512], F32, tag="qkp", name=f"vt_ps_{jt}")
        for c in range(NT):
            nc.tensor.matmul(
                ps[:, :],
                lhsT=x_sb[c][:, 128 * jt : 128 * (jt + 1)],
                rhs=wq_sb[c][:, 2 * HID : 3 * HID],
                start=(c == 0),
                stop=(c == NT - 1),
            )
        vv = vt1[jt].rearrange("p (h e) -> p h e", e=65)
        nc.vector.tensor_copy(vv[:, :, 0:64], ps.rearrange("p (h d) -> p h d", d=64))
        nc.vector.memset(vv[:, :, 64:65], 1.0)

    def emit_st_for(t, ic, jt):
        islice = slice(512 * ic, 512 * ic + 512)
        jslice = slice(128 * jt, 128 * (jt + 1))
        st = stp.tile([128, 1024], F32, tag="st", name=f"st_{t}_{ic}_{jt}")
        # the two K=64 matmuls run concurrently (PE row groups 0-1 / 2-3)
        nc.tensor.matmul(
            st[:, 0:512], lhsT=k_sb[t][0:64, jslice], rhs=q_sb[t][0:64, islice],
            start=True, stop=True,
        )
        nc.tensor.matmul(
            st[:, 512:1024], lhsT=k_sb[t][64:128, jslice],
            rhs=q_sb[t][64:128, islice],
            start=True, stop=True,
        )
        return st

    def emit_pair(t, ic, interleave, vt_jit=False, first_st=None, next_ti=None,
                  dve_jts=frozenset(), post=None, inter_from=0, last=False):
        """Attention for head pair (2t, 2t+1), i-chunk ic (512 queries).
        `interleave` closures emit independent PE work into the loop; with
        vt_jit the V^T tiles are emitted just-in-time ahead of the PV that
        first needs them. `first_st` is this pair's S^T(0) if the previous
        pair already emitted it (cross-pair pipelining); if `next_ti` is
        given, the NEXT pair's S^T(0) is emitted BEFORE the last PVs.
        h1-half exps for jt in `dve_jts` run on VectorE (2-op poly^8) with
        their PV deferred one iteration. `post` holds the PREVIOUS pair's
        deferred normalize stages. Returns (next pair's S^T(0), this pair's
        deferred normalize stages)."""
        h0, h1 = 2 * t, 2 * t + 1
        ib = 512 * ic
        islice = slice(ib, ib + 512)
        ot0 = otp.tile([65, 512], F32, tag="ot0", name=f"ot0_{t}_{ic}")
        ot1 = otp.tile([65, 512], F32, tag="ot1", name=f"ot1_{t}_{ic}")

        pv_cnt = [0, 0]

        def emit_pv(hx, jt, pt):
            pv_cnt[hx] += 1
            ot = (ot0, ot1)[hx]
            h = (h0, h1)[hx]
            vt = vt1[jt]
            nc.tensor.matmul(
                ot[:, :], lhsT=vt[:, 65 * h : 65 * h + 65], rhs=pt[:, :],
                start=(jt == 0), stop=(pv_cnt[hx] == NJ),
            )

        slot = 0
        deferred = []
        next_first = None
        sts = {0: first_st if first_st is not None else emit_st_for(t, ic, 0)}
        for jt in range(NJ):
            st = sts.pop(jt)
            use_dve = jt in dve_jts
            if use_dve:
                # ScalarE covers h0 only (so PV(h0) is ready early);
                # VectorE computes h1 via (poly4)^8 on score/8.
                pt0 = ptp.tile([128, 512], BF16, tag="pt0",
                               name=f"pt0_{t}_{ic}_{jt}")
                nc.scalar.activation(pt0[:, :], st[:, 0:512], AF.Exp, scale=8.0)
                p1 = scrp.tile([128, 512], F32, tag="p1", name=f"p1_{t}_{ic}_{jt}")
                pt1 = ptp.tile([128, 512], BF16, tag="pt1",
                               name=f"pt1_{t}_{ic}_{jt}")
                nc.vector._custom_dve(
                    EXP8_POLY, out=p1[:, :], in0=st[:, 512:1024],
                    s0=EXP_C[0], s1=EXP_C[1], imm2=EXP_C[2],
                )
                nc.vector._custom_dve(POW8, out=pt1[:, :], in0=p1[:, :])
            elif SPLIT_EXP:
                # two ScalarE instructions, h0 first: PV(h0) can issue ~550ns
                # earlier than with one full-tile exp (it waits only on the
                # h0 half). Costs ~90ns/iter of ScalarE instruction overhead.
                ptf = ptp.tile([128, 1024], BF16, tag="ptf",
                               name=f"ptf_{t}_{ic}_{jt}")
                nc.scalar.activation(ptf[:, 0:512], st[:, 0:512], AF.Exp, scale=8.0)
                nc.scalar.activation(ptf[:, 512:1024], st[:, 512:1024], AF.Exp,
                                     scale=8.0)
                pt0, pt1 = ptf[:, 0:512], ptf[:, 512:1024]
            else:
                ptf = ptp.tile([128, 1024], BF16, tag="ptf",
                               name=f"ptf_{t}_{ic}_{jt}")
                nc.scalar.activation(ptf[:, :], st[:, :], AF.Exp, scale=8.0)
                pt0, pt1 = ptf[:, 0:512], ptf[:, 512:1024]
            if jt + 1 < NJ:
                sts[jt + 1] = emit_st_for(t, ic, jt + 1)
            elif next_ti is not None:
                # cross-pair: next pair's S^T(0) goes ahead of this pair's
                # last PVs in the PE stream
                next_first = emit_st_for(next_ti[0], next_ti[1], 0)
            # V^T tiles emitted in-loop so they never gate the first exp;
            # >=2-iteration lead keeps their DVE copies off PV's critical path
            if vt_jit:
                if jt == 0:
                    emit_vt(0)
                    emit_vt(1)
                    emit_vt(2)
                elif jt + 2 < NJ:
                    emit_vt(jt + 2)
            # previous pair's deferred normalize stages, one per iteration
            if post is not None and 1 <= jt <= len(post):
                post[jt - 1]()
            # fill PE slack with independent work, spread across the loop,
            # and emitted BEFORE this iteration's PVs: PV(h0) waits on the
            # exp (~190ns on ~40% of iterations), and a proj/qk matmul
            # placed ahead of it in the in-order PE stream absorbs that
            # wait with useful work.
            # Proj groups read o2 written by the previous pair's deferred
            # muls (injected at jt 3-4), so for ic>=1 the slots start at
            # jt=5 - an interleave group emitted before its o2 writer would
            # read stale data (the framework can't wait on a future writer).
            if inter_from == 0:
                target = ((jt + 1) * len(interleave) + 11) // 12
            elif jt < inter_from:
                target = 0
            else:
                target = ((jt - inter_from + 1) * len(interleave) + 10) // 11
            while slot < min(target, len(interleave)):
                interleave[slot]()
                slot += 1
            # a DVE-produced pt1 arrives ~1us later than a ScalarE one; its
            # PV would head-of-line-block the in-order PE queue, so defer it
            # one iteration (accumulation order within ot1 is preserved).
            while deferred and deferred[0][0] <= jt - 1:
                emit_pv(1, *deferred.pop(0))
            emit_pv(0, jt, pt0)
            if use_dve or SPLIT_EXP:
                # with SPLIT_EXP every PV(h1) defers one iteration: pt1 is
                # produced second on ScalarE, so its PV would otherwise wait
                # ~600ns at the head of the in-order PE queue
                deferred.append((jt, pt1))
            else:
                emit_pv(1, jt, pt1)
        for djt, dpt in deferred:
            emit_pv(1, djt, dpt)
        # softmax normalization: divide rows 0-63 by the ones-row (64).
        # Only the psum evacuations happen here (they gate the next pair's
        # first PVs via the ot pool); den/rec/broadcast/mul are deferred
        # into the next pair via `post` - emitted here they would block the
        # next pair's DVE exps in the in-order Vector queue for ~5us.
        # reciprocal_approx_fast mis-reads non-zero partition offsets on
        # silicon, so the denominator row is staged to partition 0 first.
        if last:
            # no next pair contends for the ot psum banks: skip the o2u
            # staging copies and normalize straight out of PSUM (saves
            # ~1.3us off the end-of-kernel critical chain)
            den0 = smp.tile([1, 512], F32, tag="den0", name=f"den_{h0}_{ic}")
            nc.vector.tensor_copy(den0[:, :], ot0[64:65, :])
            rec0 = smp.tile([1, 512], F32, tag="rec0", name=f"rec_{h0}_{ic}")
            nc.vector.reciprocal_approx_fast(rec0[:, :], den0[:, :])
            den1 = smp.tile([1, 512], F32, tag="den1", name=f"den_{h1}_{ic}")
            nc.vector.tensor_copy(den1[:, :], ot1[64:65, :])
            rec1 = smp.tile([1, 512], F32, tag="rec1", name=f"rec_{h1}_{ic}")
            nc.vector.reciprocal_approx_fast(rec1[:, :], den1[:, :])
            rb0 = smp.tile([64, 512], F32, tag="rb0", name=f"rb_{h0}_{ic}")
            nc.gpsimd.partition_broadcast(rb0[:, :], rec0[:, :])
            nc.vector.tensor_mul(o2[t][0:64, islice], ot0[0:64, :], rb0[:, :])
            rb1 = smp.tile([64, 512], F32, tag="rb1", name=f"rb_{h1}_{ic}")
            nc.gpsimd.partition_broadcast(rb1[:, :], rec1[:, :])
            nc.vector.tensor_mul(o2[t][64:128, islice], ot1[0:64, :], rb1[:, :])
            return next_first, []
        o2u0 = smp.tile([65, 512], F32, tag="o2u0", name=f"o2u_{h0}_{ic}")
        nc.vector.tensor_copy(o2u0[:, :], ot0[:, :])
        o2u1 = smp.tile([65, 512], F32, tag="o2u1", name=f"o2u_{h1}_{ic}")
        nc.vector.tensor_copy(o2u1[:, :], ot1[:, :])
        den0 = smp.tile([1, 512], F32, tag="den0", name=f"den_{h0}_{ic}")
        rec0 = smp.tile([1, 512], F32, tag="rec0", name=f"rec_{h0}_{ic}")
        den1 = smp.tile([1, 512], F32, tag="den1", name=f"den_{h1}_{ic}")
        rec1 = smp.tile([1, 512], F32, tag="rec1", name=f"rec_{h1}_{ic}")
        rb0 = smp.tile([64, 512], F32, tag="rb0", name=f"rb_{h0}_{ic}")
        rb1 = smp.tile([64, 512], F32, tag="rb1", name=f"rb_{h1}_{ic}")

        def post_den0():
            nc.vector.tensor_copy(den0[:, :], o2u0[64:65, :])
            nc.vector.reciprocal_approx_fast(rec0[:, :], den0[:, :])

        def post_den1():
            nc.vector.tensor_copy(den1[:, :], o2u1[64:65, :])
            nc.vector.reciprocal_approx_fast(rec1[:, :], den1[:, :])

        def post_rb():
            # Pool runs ONLY PartitionBroadcast (op-type switches cost a
            # ~7us microcode reload on the Q7)
            nc.gpsimd.partition_broadcast(rb0[:, :], rec0[:, :])
            nc.gpsimd.partition_broadcast(rb1[:, :], rec1[:, :])

        def post_mul():
            nc.vector.tensor_mul(o2[t][0:64, islice], o2u0[0:64, :], rb0[:, :])
            nc.vector.tensor_mul(o2[t][64:128, islice], o2u1[0:64, :], rb1[:, :])

        return next_first, [post_den0, post_den1, post_rb, post_mul]

    held_proj = {}

    def emit_proj_group(o, n, c_lo=0, c_hi=NT):
        if c_lo == 0:
            ps = qkp.tile([128, 512], F32, tag="qkp", name=f"y_ps_{o}_{n}")
        else:
            ps = held_proj.pop((o, n))
        for c in range(c_lo, c_hi):
            nc.tensor.matmul(
                ps[:, :],
                lhsT=wo_sb[c][:, 128 * o : 128 * (o + 1)],
                rhs=o2[c][:, 512 * n : 512 * (n + 1)],
                start=(c == 0),
                stop=(c == NT - 1),
            )
        if c_hi < NT:
            held_proj[(o, n)] = ps
            return
        yt = ytp.tile([128, 512], F32, tag="yt", name=f"yt_{o}_{n}")
        if YT_ENGINE == "s":
            nc.scalar.activation(
                yt[:, :], ps[:, :], AF.Identity, bias=bias_sb[o][:, 0:1]
            )
        else:
            nc.vector.tensor_scalar_add(yt[:, :], ps[:, :], bias_sb[o][:, 0:1])
        nc.sync.dma_start(
            out_d[128 * o : 128 * (o + 1), 512 * n : 512 * (n + 1)], yt[:, :]
        )

    def emit_proj_partial(o, n):
        """First 3 channel-tiles of proj group (o, n); the psum tile is held
        and finished by emit_proj_group(o, n, c_lo=3) once the last pair's
        output is ready."""
        ps = qkp.tile([128, 512], F32, tag="qkp", name=f"y_ps_{o}_{n}")
        for c in range(3):
            nc.tensor.matmul(
                ps[:, :],
                lhsT=wo_sb[c][:, 128 * o : 128 * (o + 1)],
                rhs=o2[c][:, 512 * n : 512 * (n + 1)],
                start=(c == 0),
                stop=False,
            )
        held_proj[(o, n)] = ps

    # ---- emission schedule ----
    # pair 0's q (chunk 0) + full k projected up front; everything else is
    # interleaved just-in-time into earlier attention loops.
    emit_qk_group(0, 0, 0)
    emit_qk_group(0, 1, 0)

    # wo/bias loads off the critical startup path
    for c in range(NT):
        r = slice(128 * c, 128 * (c + 1))
        nc.sync.dma_start(wo_sb[c][:, :], woutT_d[r, :])
        nc.sync.dma_start(bias_sb[c][:, :], bias_d[r, :])

    # each builder returns a 1-element phase list; splitting groups into
    # two 2-matmul phases was tried (doubles the slots with PE filler) but
    # measured within noise of single-phase, which has more test mileage
    def kg(t, n):
        return [lambda: emit_qk_group(t, 1, n)]

    def qg(t, n):
        return [lambda: emit_qk_group(t, 0, n)]

    def pj(o, n):
        return [lambda: emit_proj_group(o, n)]

    # pair t's q chunk for pass ic must be emitted BEFORE its (ic, t) loop
    # (the PE executes in order - a dependency later in its own stream would
    # deadlock). q chunks for pass ic+1 therefore fire during pass ic, and
    # proj chunk n fires during pass n+1.
    inter = {
        (0, 0): kg(0, 1) + kg(0, 2) + kg(0, 3) + qg(1, 0) + kg(1, 0),
        (0, 1): kg(1, 1) + kg(1, 2) + kg(1, 3) + qg(2, 0) + kg(2, 0),
        (0, 2): kg(2, 1) + kg(2, 2) + kg(2, 3) + qg(3, 0) + kg(3, 0) + qg(0, 1),
        (0, 3): kg(3, 1) + kg(3, 2) + kg(3, 3) + qg(1, 1) + qg(2, 1) + qg(3, 1),
        (1, 0): qg(0, 2) + pj(0, 0),
        (1, 1): qg(1, 2) + pj(1, 0),
        (1, 2): qg(2, 2) + pj(2, 0),
        (1, 3): qg(3, 2) + pj(3, 0),
        (2, 0): qg(0, 3) + pj(0, 1),
        (2, 1): qg(1, 3) + pj(1, 1),
        (2, 2): qg(2, 3) + pj(2, 1),
        (2, 3): qg(3, 3) + pj(3, 1),
        (3, 0): pj(0, 2),
        (3, 1): pj(1, 2),
        (3, 2): pj(2, 2) + pj(3, 2),
        # only 2 partials fit: qkp has 2 psum banks and each held partial
        # pins one until its c=3 finisher pops it
        (3, 3): [
            lambda: emit_proj_partial(0, 3),
            lambda: emit_proj_partial(1, 3),
        ],
    }
    seq = [(ic, t) for ic in range(4) for t in range(NT)]
    pending_st = None
    pending_post = None
    for i, (ic, t) in enumerate(seq):
        nxt = seq[i + 1] if i + 1 < len(seq) else None
        # no DVE offload in the very first pair (its DVE queue is busy with
        # JIT V^T evacuations) and none on jt 15 (enforced in DVE_JTS)
        dj = frozenset() if (ic == 0 and t == 0) else DVE_JTS
        pending_st, pending_post = emit_pair(
            t, ic, inter.get((ic, t), []),
            vt_jit=(ic == 0 and t == 0),
            first_st=pending_st,
            next_ti=(nxt[1], nxt[0]) if nxt else None,
            dve_jts=dj,
            post=pending_post,
            # 5 measured best (7 was tried for the ~1us/pair o2-mul wait
            # of the jt5 proj group, but pushing proj work later loses
            # more overlap than the wait costs)
            inter_from=0 if ic == 0 else 5,
            last=(i + 1 == len(seq)),
        )
    for p in pending_post:
        p()
    emit_proj_group(0, 3, c_lo=3)
    emit_proj_group(1, 3, c_lo=3)
    emit_proj_group(2, 3)
    emit_proj_group(3, 3)
    ctx.close()


_COMPILED = None


def _get_compiled():
    global _COMPILED
    if _COMPILED is None:
        nc = bacc.Bacc(
            "TRN2", target_bir_lowering=False, debug=False, num_devices=NCORES
        )
        x_d = nc.dram_tensor("x", [C, L], BF16, kind="ExternalInput").ap()
        wqkvT_d = nc.dram_tensor("wqkvT", [C, 3 * HID], BF16, kind="ExternalInput").ap()
        woutT_d = nc.dram_tensor("woutT", [HID, C], BF16, kind="ExternalInput").ap()
        bias_d = nc.dram_tensor("bias", [C, 1], F32, kind="ExternalInput").ap()
        out_d = nc.dram_tensor("out", [C, L], F32, kind="ExternalOutput").ap()
        with tile.TileContext(nc) as tc:
            build_kernel(tc, out_d, x_d, wqkvT_d, woutT_d, bias_d)
        nc.compile()
        _COMPILED = nc
    return _COMPILED


def make_in_maps(x, w_qkv, w_out, b_out):
    xb = np.asarray(x, dtype=np.float32).astype(ml_dtypes.bfloat16)
    wq_f = np.asarray(w_qkv, dtype=np.float32).T.copy()
    wq_f[:, 0:HID] *= SCALE / 8.0  # exp scale folded into the q projection
    wqkvT = np.ascontiguousarray(wq_f.astype(ml_dtypes.bfloat16))
    woutT = np.ascontiguousarray(
        np.asarray(w_out, dtype=np.float32).T.astype(ml_dtypes.bfloat16)
    )
    bias = np.ascontiguousarray(np.asarray(b_out, dtype=np.float32).reshape(C, 1))
    return [
        {
            "x": np.ascontiguousarray(xb[b]),
            "wqkvT": wqkvT,
            "woutT": woutT,
            "bias": bias,
        }
        for b in range(B)
    ]


LAST_RESULTS = None


def _install_ntff_hook():
    """Provide antenv.axon_hooks (absent from this image) so trace=True works."""
    import types

    try:
        from antenv.axon_hooks import get_axon_ntff_profile_hook  # noqa: F401

        return
    except ImportError:
        pass
    sys.path.insert(0, "/root/.axon_site")
    from trn_agent_boot.trn_boot import _ntff_profile_via_ctypes

    hook = _ntff_profile_via_ctypes("/opt/axon/libaxon_pjrt.so")
    import antenv

    mod = types.ModuleType("antenv.axon_hooks")
    mod._hook = hook
    mod.get_axon_ntff_profile_hook = lambda: mod._hook
    mod.set_axon_ntff_profile_hook = lambda h: setattr(mod, "_hook", h)
    sys.modules["antenv.axon_hooks"] = mod
    antenv.axon_hooks = mod
    # artifact upload has no egress in this container - make it a no-op
    bass_utils.upload_artifacts = lambda tmpdir: tmpdir


def kernel(x, w_qkv, w_out, b_out):
    global LAST_RESULTS
    nc = _get_compiled()
    in_maps = make_in_maps(x, w_qkv, w_out, b_out)
    trace = bool(int(os.environ.get("KERNEL_TRACE", "0")))
    if trace:
        _install_ntff_hook()
    res = bass_utils.run_bass_kernel_spmd(
        nc, in_maps, core_ids=list(range(NCORES)), trace=trace
    )
    LAST_RESULTS = res
    out = np.stack([np.asarray(res.results[b]["out"]) for b in range(B)])
    return out.astype(np.float32)
